# revision 46
# baseline (speedup 1.0000x reference)
"""Trainium2 Bass kernel for single-head causal attention (nn_DefaultAttention).

Reference computation (B=4, S=2048, E=1024, fp32):
    k = x @ Wk.T + bk ; q = x @ Wq.T + bq ; v = x @ Wv.T + bv
    sim[b,s,t] = k[b,s]·q[b,t] / sqrt(E), masked to t<=s
    out[b,s]   = softmax_t(sim[b,s,:]) @ v[b,:]

Algebraic folding (single head => the QK / V weight folds are exact):
    sim*sqrt(E) = x (Wk^T Wq) x^T + terms;  the per-s and const bias terms
    are constant along the softmax axis t and cancel; the per-t term
    (Wq^T bk)·x[t] folds into a per-feature bias c on z := x M
    (M = Wk^T Wq, host-computed).  So
        scores[s,t] = (x[s] M + c) · x[t]
    and the q/k projections never run on-device.  Likewise
        out = P @ v / den = (P @ x) @ Wv^T / den + bv
    so the v projection becomes a post-GEMM on P@x.  Per-core work drops
    from 2.5 projection-equivalents + attention to 1.0 + attention.

fp8 DoubleRow residual arithmetic: every operand of the z / scores / u^T
GEMMs is split into hi + lo fp8 parts (e4m3 for x/M/z, e5m2 for P whose
range exceeds e4m3), and products are computed with the 3-term expansion
hi*hi + hi*lo + lo*hi (lo*lo ~ eps^2 dropped).  DoubleRow packs 2
contraction tiles per instruction at 0.5 cycles/row, so the 3-term fp8
GEMM costs 0.75x of its bf16 version while being *more* accurate
(residual quantization error ~eps^2).  The final (u/den) @ Wv^T GEMM
stays bf16: its fp8 savings are small and u would need an extra
normalize+split round trip.

Sharding: 8 cores = 4 batches x 2 interleaved 128-row query-block sets.
Slot k=0..7 processes a uniform T=2k+2 key tiles against the query block
at permuted position 2k; per-core 0/1 masks zero invalid t>s entries
after exp (softmax's shift-invariance disposes of the per-s bias terms).

Dataflow per core (PSUM fp32 accumulation everywhere):
    z^T[e',s]  = sum_e M32[e,e'] x[s,e] / 32 + c[e']     (ACT bias+scale,
                 split into e4m3 hi/lo: ACT hi, ACT tmp, DVE lo)
    ps[t,s]    = sum_e x^T[e,t] z^T[e,s]                 (3-term fp8)
    Pb[t,s]    = exp(ps/sqrt(E)) * mask  (ACT+DVE) -> e5m2 hi/lo
    uT[e,s]    = sum_t xrow[t,e] P[t,s]  (3-term fp8), den via ones moving
    out[s,f]   = (sum_e uT[e,s] wvT[e,f]) * (1/den) + bv (bf16 GEMM; ACT
                 per-partition reciprocal scale; DVE bias add)
"""

import numpy as np


def _ensure_concourse():
    try:
        import concourse  # noqa: F401
    except ImportError:
        import sys
        for p in ("/opt/trn_rl_repo", "/root/.axon_site/_ro/trn_rl_repo"):
            if p not in sys.path:
                sys.path.append(p)
        import concourse  # noqa: F401


E = 1024
S = 2048
B = 4
NCORES = 8
ET = E // 128    # 8 feature tiles
ST = S // 128    # 16 key tiles
SCALE = 1.0 / np.sqrt(np.float32(E))
SLOT_T = (2, 4, 6, 8, 10, 12, 14, 16)   # key-128-tiles per slot (uniform)
SRC = tuple(256 * k for k in range(8))   # permuted query-col base per slot
MBASE = (0, 2, 6, 12, 20, 30, 42, 56)    # flat mask index base per slot
NMASK = 72
SLOT_ORDER = (0, 1, 2, 3, 4, 5, 6, 7)    # ascending: xt/xr/mask demand
                                         # becomes incremental, giving the
                                         # DMA stream slack everywhere
# 128-row-block permutations: slot k (T=2k+2) hosts the core's query block
# with causal need n_k (A: {1,4,5,8,9,12,13,16}, B: {2,3,6,7,10,11,14,15})
# at position 2k; every block's needed key tiles land within [0, T_k).
PERM_BLOCKS = {
    0: [0, 1, 3, 2, 4, 5, 7, 6, 8, 9, 11, 10, 12, 13, 15, 14],
    1: [1, 0, 2, 3, 5, 4, 6, 7, 9, 8, 10, 11, 13, 12, 14, 15],
}
XA = 1024                      # xrow free width (16B-aligned pair
                               # stride required by dual-fp8 LdWeights)

_CACHE = {}
import os as _os
_PROBE = _os.environ.get("KERNEL_PROBE", "")  # '', 'z', 'zs', 'zsu'


def _build_program():
    _ensure_concourse()
    from contextlib import ExitStack
    import concourse.tile as tile
    import concourse.bass as bass
    from concourse import bacc, mybir

    F32 = mybir.dt.float32
    BF16 = mybir.dt.bfloat16
    F8 = mybir.dt.float8e4
    F8E5 = mybir.dt.float8e5
    DR = mybir.MatmulPerfMode.DoubleRow
    ts = bass.ts
    Exp = mybir.ActivationFunctionType.Exp
    Ident = mybir.ActivationFunctionType.Identity

    nc = bacc.Bacc("TRN2", target_bir_lowering=False, debug=False)

    xTh = nc.dram_tensor("xTh", [E, S], F8, kind="ExternalInput").ap()
    xTl = nc.dram_tensor("xTl", [E, S], F8, kind="ExternalInput").ap()
    xrowh = nc.dram_tensor("xrowh", [128, ST, XA], F8, kind="ExternalInput").ap()
    xrowl = nc.dram_tensor("xrowl", [128, ST, XA], F8, kind="ExternalInput").ap()
    # hi/lo channel-interleaved ([..., 2]) so narrow column slices still
    # move as >=512B contiguous runs (full DMA rate).
    mwi = nc.dram_tensor("mwi", [E, E, 2], F8, kind="ExternalInput").ap()
    xtqi = nc.dram_tensor("xtqi", [E, 1024, 2], F8, kind="ExternalInput").ap()
    wvT = nc.dram_tensor("wvT", [E, E], BF16, kind="ExternalInput").ap()
    zbp = nc.dram_tensor("zbp", [128, ET], F32, kind="ExternalInput").ap()
    bv = nc.dram_tensor("bv", [E], BF16, kind="ExternalInput").ap()
    # masks: [128, NMASK*128] p-major so each slot's block is contiguous
    # along the free dim (single descriptor per partition row).
    masks = nc.dram_tensor("masks", [128, NMASK * 128], F8, kind="ExternalInput").ap()
    out_sd = nc.dram_tensor("out_sd", [1024, E], BF16, kind="ExternalOutput").ap()

    with tile.TileContext(nc) as tc, ExitStack() as top:
        # ---- persistent smalls -------------------------------------------
        smalls = top.enter_context(tc.tile_pool(name="smalls", bufs=1))
        zb_sb = smalls.tile([128, ET], F32)
        nc.sync.dma_start(out=zb_sb, in_=zbp)
        bvb = smalls.tile([128, E], BF16)
        bv_bcast = bass.AP(tensor=bv.tensor, offset=bv.offset,
                           ap=[[0, 128]] + list(bv.ap))
        ones2 = smalls.tile([128, 2, 1], F8E5)
        nc.vector.memset(ones2, 1.0)

        # Warm the ACT function tables (Identity/Exp) up front so the
        # LoadActFuncSet DMA doesn't queue behind the bulk loads later.
        scratch = smalls.tile([1, 8], F32)
        nc.vector.memset(scratch, 0.0)
        nc.scalar.activation(scratch, scratch, Ident, bias=0.0, scale=1.0)
        nc.scalar.activation(scratch, scratch, Exp, scale=1.0)

        # ---- big persistent operands -------------------------------------
        big = top.enter_context(tc.tile_pool(name="big", bufs=1))
        xth = big.tile([128, ET, S], F8)        # x^T permuted hi
        xtl = big.tile([128, ET, S], F8)        # x^T permuted lo
        xrh = big.tile([128, ST, XA], F8)       # x row-major, hi
        xrl = big.tile([128, ST, XA], F8)       # x row-major, lo
        wv = big.tile([128, ET, E], BF16)       # Wv^T as [e, f]
        zth = big.tile([128, ET, 1024], F8)     # z^T hi
        ztl = big.tile([128, ET, 1024], F8)     # z^T lo
        mk = big.tile([128, NMASK, 128], F8)    # all causal masks

        # mw hi/lo only live through phase Z; scoped pool.
        zin_ctx = tc.tile_pool(name="zin", bufs=1, side="right")
        zin = zin_ctx.__enter__()
        mwt = zin.tile([128, ET, E, 2], F8)
        xtq = zin.tile([128, ET, 1024, 2], F8)

        def load_rearr(eng, dst, src, db, width=512):
            eng.dma_start(
                out=dst[:, :, ts(db, width)],
                in_=src.rearrange("(e p) s -> p e s", p=128)[:, :, ts(db, width)],
            )

        def load_i(dst, src, lo, width):
            # interleaved [*, 2] tensors: [128, ET, width, 2]
            nc.sync.dma_start(
                out=dst[:, :, lo:lo + width, :],
                in_=src.rearrange("(e p) s c -> p e s c", p=128)[
                    :, :, lo:lo + width, :],
            )

        # All transfers serialize on the shared DMA-engines device
        # (~344 GB/s): what matters is total bytes and consumption order.
        # Everything rides the SP queue: ACT-queue DMAs would steal
        # Activation SEQ time from exp/Ident dispatch.  Phase Z's first
        # chains (slot pair 6,7 x ep ascending) unblock after the first
        # two 0.5 MB pieces.
        def load_xr(jc):
            nc.sync.dma_start(out=xrh[:, 4 * jc:4 * jc + 4, :],
                              in_=xrowh[:, 4 * jc:4 * jc + 4, :])
            nc.sync.dma_start(out=xrl[:, 4 * jc:4 * jc + 4, :],
                              in_=xrowl[:, 4 * jc:4 * jc + 4, :])

        def load_i_e(dst, src, elo, ehi, lo, width):
            nc.sync.dma_start(
                out=dst[:, elo:ehi, lo:lo + width, :],
                in_=src.rearrange("(e p) s c -> p e s c", p=128)[
                    :, elo:ehi, lo:lo + width, :],
            )

        load_i_e(xtq, xtqi, 0, 4, 0, 256)   # slots 0,1 x contraction e0-3
        load_i_e(mwt, mwi, 0, 4, 0, 256)    # ep 0,1 x e0-3
        load_i_e(xtq, xtqi, 4, 8, 0, 256)
        load_i_e(mwt, mwi, 4, 8, 0, 256)
        load_i(mwt, mwi, 256, 256)     # ep 2,3
        load_i(mwt, mwi, 512, 256)     # ep 4,5
        load_i(mwt, mwi, 768, 256)     # ep 6,7
        load_i(xtq, xtqi, 256, 256)    # slots 2,3
        load_i(xtq, xtqi, 512, 512)    # slots 4..7
        # remaining loads by first-consumption time (ascending slots make
        # every deadline loose):
        load_rearr(nc.sync, xth, xTh, 0, 512)
        load_rearr(nc.sync, xtl, xTl, 0, 512)
        load_xr(0)
        nc.sync.dma_start(out=mk, in_=masks.rearrange("p (j m) -> p j m", m=128))
        load_xr(1)
        load_rearr(nc.sync, xth, xTh, 1, 512)
        load_rearr(nc.sync, xtl, xTl, 1, 512)
        load_rearr(nc.sync, wv, wvT, 0, 512)
        load_rearr(nc.sync, wv, wvT, 1, 512)
        nc.sync.dma_start(out=bvb, in_=bv_bcast)
        load_rearr(nc.sync, xth, xTh, 2, 512)
        load_rearr(nc.sync, xtl, xTl, 2, 512)
        load_xr(2)
        load_rearr(nc.sync, xth, xTh, 3, 512)
        load_rearr(nc.sync, xtl, xTl, 3, 512)
        load_xr(3)

        def mm3(out_ap, stat_h, stat_l, mov_h, mov_l, npair, first, last):
            """3-term fp8 residual product into one PSUM accumulation
            chain: hh, hl, lh — each as DoubleRow pairs over npair
            contraction-tile pairs.  stat_*/mov_* map pair index ->
            [128, 2, N] APs."""
            seq = [(stat_h, mov_h), (stat_h, mov_l), (stat_l, mov_h)]
            n = 3 * npair
            i = 0
            for p in range(npair):
                for sh, mv in seq:
                    nc.tensor.matmul(
                        out_ap, sh(p), mv(p),
                        start=(first and i == 0),
                        stop=(last and i == n - 1),
                        perf_mode=DR,
                    )
                    i += 1

        # ---- phase Z: z^T = (x M32)/32 + c, local queries, split hi/lo ---
        proj_ctx = tc.tile_pool(name="pz", bufs=7, space="PSUM")
        proj_pool = proj_ctx.__enter__()
        ztm_ctx = tc.tile_pool(name="ztm", bufs=5)
        ztm_pool = ztm_ctx.__enter__()
        QPOS = {k: k for k in range(8)}
        Mul = mybir.AluOpType.mult
        Add = mybir.AluOpType.add
        for spair in ((0, 1), (2, 3), (4, 5), (6, 7)):
            for ep in range(ET):
                pz = proj_pool.tile([128, 2, 128], F32, tag="pz")
                for i, sg in enumerate(spair):
                    q0 = 128 * QPOS[sg]
                    mm3(
                        pz[:, i, :],
                        lambda p: mwt[:, 2 * p:2 * p + 2, ts(ep, 128), 0:1],
                        lambda p: mwt[:, 2 * p:2 * p + 2, ts(ep, 128), 1:2],
                        lambda p, q0=q0: xtq[:, 2 * p:2 * p + 2,
                                             q0:q0 + 128, 0:1],
                        lambda p, q0=q0: xtq[:, 2 * p:2 * p + 2,
                                             q0:q0 + 128, 1:2],
                        npair=ET // 2, first=True, last=True,
                    )
                c0 = 128 * spair[0]
                nc.scalar.activation(zth[:, ep, c0:c0 + 256], pz, Ident,
                                     bias=zb_sb[:, ep:ep + 1],
                                     scale=float(1.0 / 32.0))
                ztm = ztm_pool.tile([128, 2, 128], BF16, tag="ztm")
                zbb = bass.AP(tensor=zb_sb.tensor,
                              offset=zb_sb[:, ep:ep + 1].offset,
                              ap=[list(zb_sb.ap[0]), [0, 2], [0, 128]])
                nc.vector.scalar_tensor_tensor(ztm, pz, float(1.0 / 32.0),
                                               zbb, Mul, Add)
                nc.gpsimd.tensor_sub(ztl[:, ep, c0:c0 + 256], ztm,
                                     zth[:, ep, c0:c0 + 256])
        ztm_ctx.__exit__(None, None, None)
        proj_ctx.__exit__(None, None, None)
        zin_ctx.__exit__(None, None, None)

        # ---- fused scores -> P -> u^T -> final GEMM, per slot ------------
        # PSUM budget (8 banks): ps pairs 3 + pu 2 + pd 1 + po 2.
        p_pool = top.enter_context(tc.tile_pool(name="pP", bufs=1))
        p_hi = {}
        p_lo = {}

        with tc.tile_pool(name="osb", bufs=3) as osb_pool, \
             tc.tile_pool(name="ut", bufs=2) as ut_pool, \
             tc.tile_pool(name="denr", bufs=2) as denr_pool, \
             tc.tile_pool(name="pb", bufs=5) as pb_pool, \
             tc.tile_pool(name="ps", bufs=3, space="PSUM") as ps_pool, \
             tc.tile_pool(name="pu", bufs=2, space="PSUM") as pu_pool, \
             tc.tile_pool(name="pd", bufs=1, space="PSUM") as pd_pool, \
             tc.tile_pool(name="po", bufs=2, space="PSUM") as po_pool:

            ut_tiles = {}

            def emit_scores(sg):
                # j-pairs share one PSUM bank; chains are emitted
                # sequentially so the pending-zero region flip is safe.
                T = SLOT_T[sg]
                for jp in range(T // 2):
                    ps = ps_pool.tile([128, 2, 128], F32, tag="ps",
                                      name=f"ps_{sg}_{jp}")
                    for jj in range(2):
                        j = 2 * jp + jj
                        mm3(
                            ps[:, jj, :],
                            lambda p, j=j: xth[:, 2 * p:2 * p + 2, ts(j, 128)],
                            lambda p, j=j: xtl[:, 2 * p:2 * p + 2, ts(j, 128)],
                            lambda p: zth[:, 2 * p:2 * p + 2, ts(sg, 128)],
                            lambda p: ztl[:, 2 * p:2 * p + 2, ts(sg, 128)],
                            npair=ET // 2, first=True, last=True,
                        )
                    Pb = pb_pool.tile([128, 2, 128], BF16, tag="pb",
                                      name=f"pb_{sg}_{jp}")
                    nc.scalar.activation(Pb, ps, Exp, scale=float(SCALE))
                    nc.vector.tensor_mul(Pb, Pb,
                                         mk[:, MBASE[sg] + 2 * jp:
                                            MBASE[sg] + 2 * jp + 2, :])
                    Ph = p_pool.tile([128, 2, 128], F8E5, tag=f"Ph{jp}",
                                     name=f"Ph_{sg}_{jp}", bufs=4)
                    nc.scalar.copy(Ph, Pb)
                    Pl = p_pool.tile([128, 2, 128], F8E5, tag=f"Pl{jp}",
                                     name=f"Pl_{sg}_{jp}", bufs=4)
                    nc.vector.tensor_sub(Pl, Pb, Ph)
                    p_hi[(sg, jp)] = Ph
                    p_lo[(sg, jp)] = Pl

            def emit_ut(sg):
                """u^T[e, s-slot] = sum_j xrow[t,e-chunk]^T @ P_j (raw,
                unnormalized); den[s] via ones-moving matmul chain."""
                T = SLOT_T[sg]
                npair = T // 2
                pd = pd_pool.tile([128, 1], F32, tag="pd", name=f"pd_{sg}")
                ut = ut_pool.tile([128, ET, 128], BF16, tag="ut",
                                  name=f"ut_{sg}")
                for w in range(2):
                    pu = pu_pool.tile([128, 4, 128], F32, tag="pu",
                                      name=f"pu_{sg}_{w}")
                    for ei in range(4):
                        et = 4 * w + ei
                        mm3(
                            pu[:, ei, :],
                            lambda p, et=et: xrh[:, 2 * p:2 * p + 2,
                                                 ts(et, 128)],
                            lambda p, et=et: xrl[:, 2 * p:2 * p + 2,
                                                 ts(et, 128)],
                            lambda p: p_hi[(sg, p)],
                            lambda p: p_lo[(sg, p)],
                            npair=npair, first=True, last=True,
                        )
                    for ei in range(4):
                        nc.vector.tensor_copy(ut[:, 4 * w + ei, :],
                                              pu[:, ei, :])
                for jp in range(npair):
                    nc.tensor.matmul(pd, p_hi[(sg, jp)], ones2,
                                     start=(jp == 0), stop=False,
                                     perf_mode=DR)
                for jp in range(npair):
                    nc.tensor.matmul(pd, p_lo[(sg, jp)], ones2,
                                     start=False, stop=(jp == npair - 1),
                                     perf_mode=DR)
                dr = denr_pool.tile([128, 1], F32, tag="dr", name=f"dr_{sg}")
                nc.vector.reciprocal(dr, pd)
                ut_tiles[sg] = (ut, dr)

            def emit_final(sg, last=False):
                rows = 128 * sg
                ut, dr = ut_tiles[sg]
                for fb in range(2):
                    po = po_pool.tile([128, 512], F32, tag="po",
                                      name=f"po_{sg}_{fb}")
                    for et in range(ET):
                        nc.tensor.matmul(
                            po, ut[:, et, :], wv[:, et, ts(fb, 512)],
                            start=(et == 0), stop=(et == ET - 1),
                        )
                    osb = osb_pool.tile([128, 512], BF16, tag="osb",
                                        name=f"osb_{sg}_{fb}")
                    # out = (uT @ wv) * (1/den)  (per-partition scale), + bv
                    # The very last slot drains in 128-col chunks so the
                    # ACT->DVE->DMA tail pipelines instead of serializing.
                    nch = 1
                    w = 512 // nch
                    for ch in range(nch):
                        sl = slice(ch * w, ch * w + w)
                        nc.scalar.activation(osb[:, sl], po[:, sl], Ident,
                                             scale=dr)
                        nc.vector.tensor_add(osb[:, sl], osb[:, sl],
                                             bvb[:, 512 * fb + ch * w:
                                                 512 * fb + ch * w + w])
                        nc.sync.dma_start(
                            out=out_sd[rows:rows + 128,
                                       512 * fb + ch * w:
                                       512 * fb + ch * w + w],
                            in_=osb[:, sl])

            # Pipeline: scores(s_a) ahead of ut(s_a) ahead of final(s_a),
            # with the next slot's scores interleaved so PE never waits on
            # ACT/DVE.
            order = list(SLOT_ORDER)
            if _PROBE == "z":
                order = []
            if order:
                emit_scores(order[0])
                if len(order) > 1:
                    emit_scores(order[1])
            for idx, sg in enumerate(order):
                if idx + 2 < len(order):
                    emit_scores(order[idx + 2])
                if _PROBE == "zs":
                    continue
                emit_ut(sg)
                if _PROBE == "zsu":
                    continue
                if idx >= 1:
                    emit_final(order[idx - 1])
            if order and _PROBE == "":
                emit_final(order[-1], last=True)

    nc.compile()
    return nc


def _get_program():
    if "nc" not in _CACHE:
        _CACHE["nc"] = _build_program()
    return _CACHE["nc"]


def _perm_indices(h):
    return np.concatenate(
        [np.arange(128 * b, 128 * (b + 1)) for b in PERM_BLOCKS[h]])


def _split8(a, dt):
    hi = np.asarray(a, dtype=dt)
    lo = np.asarray(np.asarray(a, np.float32) - hi.astype(np.float32),
                    dtype=dt)
    return hi, lo


def _host_prep(x, Wk, bk, Wq, bq, Wv, bv):
    """Build per-core in_maps (fp8 hi/lo + bf16 streams)."""
    import ml_dtypes
    f32 = np.float32
    bf16 = ml_dtypes.bfloat16
    f8 = ml_dtypes.float8_e4m3

    Wk64 = np.asarray(Wk, np.float64)
    Wq64 = np.asarray(Wq, np.float64)
    M32 = np.ascontiguousarray((Wk64.T @ Wq64) * 32.0).astype(f32)
    mwhc, mwlc = _split8(M32, f8)
    mwic = np.ascontiguousarray(np.stack([mwhc, mwlc], axis=-1))
    zb = (np.asarray(Wq, f32).T @ np.asarray(bk, f32))  # [E] per-e' bias on z
    zbpv = np.ascontiguousarray(zb.reshape(ET, 128).T)
    wvTc = np.ascontiguousarray(np.asarray(Wv, f32).T).astype(bf16)
    bvc = np.ascontiguousarray(np.asarray(bv, f32)).astype(bf16)

    in_maps = []
    for c in range(NCORES):
        b, h = divmod(c, 2)
        perm = _perm_indices(h)
        xb = np.asarray(x[b], f32)
        xTh_, xTl_ = _split8(np.ascontiguousarray(xb.T[:, perm]), f8)
        # local-query cols in z-consumption order (ascending slots)
        qorder = (0, 1, 2, 3, 4, 5, 6, 7)
        qcols = np.concatenate([perm[SRC[sg]:SRC[sg] + 128] for sg in qorder])
        xqh_, xql_ = _split8(np.ascontiguousarray(xb.T[:, qcols]), f8)
        xtqic = np.ascontiguousarray(np.stack([xqh_, xql_], axis=-1))
        # xrow: [128, ST, XA] — x rows in permuted order.
        xrowb = np.ascontiguousarray(
            xb[perm, :].reshape(ST, 128, XA).transpose(1, 0, 2))
        xrh_, xrl_ = _split8(xrowb, f8)
        m = np.zeros((NMASK, 128, 128), f32)
        for sg in range(8):
            s_g = perm[SRC[sg]:SRC[sg] + 128]
            for j in range(SLOT_T[sg]):
                t_g = perm[128 * j:128 * (j + 1)]
                m[MBASE[sg] + j] = (t_g[:, None] <= s_g[None, :]).astype(f32)
        mp = np.ascontiguousarray(
            m.transpose(1, 0, 2).reshape(128, NMASK * 128))
        in_maps.append({
            "xTh": xTh_, "xTl": xTl_, "xrowh": xrh_, "xrowl": xrl_,
            "mwi": mwic, "xtqi": xtqic, "wvT": wvTc, "zbp": zbpv, "bv": bvc,
            "masks": mp.astype(f8),
        })
    return in_maps


def _assemble(results):
    out = np.empty((B, S, E), np.float32)
    for c in range(NCORES):
        b, h = divmod(c, 2)
        perm = _perm_indices(h)
        osd = np.asarray(results[c]["out_sd"], np.float32)
        for sg in range(8):
            rows = perm[SRC[sg]:SRC[sg] + 128]
            out[b, rows, :] = osd[128 * sg:128 * (sg + 1), :]
    return out


def kernel(x, Wk, bk, Wq, bq, Wv, bv):
    _ensure_concourse()
    from concourse.bass_utils import run_bass_kernel_spmd
    nc = _get_program()
    in_maps = _host_prep(x, Wk, bk, Wq, bq, Wv, bv)
    res = run_bass_kernel_spmd(nc, in_maps, list(range(NCORES)))
    return _assemble(res.results)
